# revision 2
# baseline (speedup 1.0000x reference)
"""Trainium2 Bass kernel for a rate-1/2, constraint-length-3 feedforward
convolutional encoder (generator polynomials "101" and "111", MSB-first).

The trellis scan in the reference collapses to elementwise XORs of shifted
input bits (zero initial state):

    out0[t] = u[t] ^ u[t-2]            (poly "101")
    out1[t] = u[t] ^ u[t-1] ^ u[t-2]   (poly "111")

with the codeword interleaved time-major: y[:, 2t] = out0[t], y[:, 2t+1] = out1[t].

All values are exactly 0/1, so the kernel moves single bytes instead of
f32: the host casts the f32 input to uint8 (exact), the device XORs bytes,
and the uint8 output is cast back to f32 on the host. That cuts HBM
traffic per core from 24 MiB to 6 MiB (2 in + 4 out), which is the
binding roofline at ~360 GB/s/core.

Sharding: pure data parallel over the batch dim across 8 NeuronCores.
"""

import numpy as np

N_CORES = 8
B, K = 8192, 2048
N_OUT = 2
SHARD_B = B // N_CORES  # 1024 codewords per core
P = 128                 # SBUF partitions

_compiled = {}


def _build_nc():
    import concourse.bass as bass  # noqa: F401
    import concourse.tile as tile
    from concourse import bacc, mybir

    nc = bacc.Bacc(
        "TRN2",
        target_bir_lowering=False,
        debug=False,
        enable_asserts=False,
    )
    x = nc.dram_tensor("x", [SHARD_B, K], mybir.dt.uint8, kind="ExternalInput").ap()
    y = nc.dram_tensor(
        "y", [SHARD_B, N_OUT * K], mybir.dt.uint8, kind="ExternalOutput"
    ).ap()

    n_groups = SHARD_B // P  # 8 row-groups of 128
    N_SLOTS = 6

    with tile.TileContext(nc) as tc:
        with (
            tc.tile_pool(name="xin", bufs=1) as in_pool,
            tc.tile_pool(name="out", bufs=5) as out_pool,
        ):
            # Persistent input slots with 2 leading zero columns so the
            # shifted views u[t-1], u[t-2] fall out of plain column offsets.
            # Zero columns are written once; per-iteration DMAs only write
            # cols [2:].
            in_slots = [
                in_pool.tile(
                    [P, K + 4], mybir.dt.uint8, tag=f"xin{j}", name=f"xin{j}"
                )
                for j in range(N_SLOTS)
            ]
            for j in range(N_SLOTS):
                nc.vector.memset(in_slots[j][:, 0:2], 0)

            for g in range(n_groups):
                xin = in_slots[g % N_SLOTS]
                rows = slice(g * P, (g + 1) * P)
                nc.sync.dma_start(xin[:, 2 : 2 + K], x[rows, :])

                a = xin[:, 2 : 2 + K]  # u[t]
                b = xin[:, 1 : 1 + K]  # u[t-1]
                c = xin[:, 0:K]        # u[t-2]

                out = out_pool.tile(
                    [P, N_OUT * K], mybir.dt.uint8, tag="out", name="out"
                )
                even = out[:, 0 : N_OUT * K : 2]
                odd = out[:, 1 : N_OUT * K : 2]

                nc.vector.tensor_tensor(even, a, c, mybir.AluOpType.bitwise_xor)
                nc.vector.tensor_tensor(odd, even, b, mybir.AluOpType.bitwise_xor)

                # Output DMAs on the SWDGE path (GpSimd sequencer) so input
                # and output streams issue independently.
                nc.gpsimd.dma_start(y[rows, :], out[:])

    nc.compile()
    return nc


def _get_nc():
    if "nc" not in _compiled:
        _compiled["nc"] = _build_nc()
    return _compiled["nc"]


def kernel(**inputs) -> np.ndarray:
    from concourse.bass_utils import run_bass_kernel_spmd

    x_full = np.ascontiguousarray(
        np.asarray(inputs["inputs"]).astype(np.uint8)
    )
    assert x_full.shape == (B, K), x_full.shape

    nc = _get_nc()
    in_maps = [
        {"x": x_full[i * SHARD_B : (i + 1) * SHARD_B]} for i in range(N_CORES)
    ]
    res = run_bass_kernel_spmd(nc, in_maps, core_ids=list(range(N_CORES)))
    out = np.concatenate([r["y"] for r in res.results], axis=0)
    return out.astype(np.float32)


# revision 5
# speedup vs baseline: 1.4036x; 1.4036x over previous
"""Trainium2 Bass kernel for a rate-1/2, constraint-length-3 feedforward
convolutional encoder (generator polynomials "101" and "111", MSB-first).

The trellis scan in the reference collapses to elementwise XORs of shifted
input bits (zero initial state):

    out0[t] = u[t] ^ u[t-2]            (poly "101")
    out1[t] = u[t] ^ u[t-1] ^ u[t-2]   (poly "111")

with the codeword interleaved time-major: y[:, 2t] = out0[t], y[:, 2t+1] = out1[t].

All values are exactly 0/1, so the kernel moves single bytes instead of
f32 (host casts f32<->u8, exact): 6 MiB of HBM traffic per core instead
of 24 MiB — the binding roofline at ~360 GB/s/core.

Compute runs as two custom DVE ops over uint16 *byte-pair* views of the
input (v = x[2i] + 256*x[2i+1], w = x[2i-2] + 256*x[2i-1]):

    even op: out16[2i]   = e0 + 256*(e0^c1)   (bytes E[2i], O[2i])
    odd  op: out16[2i+1] = e1 + 256*(e1^a0)   (bytes E[2i+1], O[2i+1])

where d = |v-w| gives e0 = parity(d) = (d!=0)(d!=256) and e1 = (d>=255);
a0 = parity(v); c1 = (w>=256). Each op emits two interleaved output
bytes per element — 2 bytes/cycle/lane — so the DVE (~17 us) hides
under the DMA roofline. The DVE custom-op datapath is fp32: IS_NE /
ABS_DIFF / compares give exact {0,1} logic (verified bit-exact on HW).

Layout: 2 consecutive batch rows per partition -> 4 KiB contiguous HBM
reads and 8 KiB writes per descriptor. Sharding: pure data parallel
over the batch dim across 8 NeuronCores.
"""

import numpy as np

N_CORES = 8
B, K = 8192, 2048
N_OUT = 2
SHARD_B = B // N_CORES   # 1024 codewords per core
P = 128                  # SBUF partitions
RPP = 2                  # rows per partition
ROWS_G = P * RPP         # 256 rows per group
N_GROUPS = SHARD_B // ROWS_G  # 4
CHUNK = K + 2            # 2050 bytes: [0, 0, row...]

_compiled = {}


def _register_ops():
    import concourse.dve_ops as dve_ops
    from concourse.dve_table_gen import dve_ver_for
    from concourse.dve_spec import (
        Spec, Src0, Src1, C0, C1, C2, AluOp, Bin, Zero, lower, _has_src1,
    )
    from concourse.dve_uop import DveOpSpec

    def register(name, spec):
        if name in dve_ops._SUB_OPCODE_FOR_NAME:
            return next(op for op in dve_ops.OPS if op.name == name)
        row = dve_ops._CUSTOM_DVE_ROW_BASE + len(dve_ops.OPS)
        assert row < 0x20, "custom-DVE opcode row overflow"
        dve_ops._SUB_OPCODE_FOR_NAME[name] = row
        ver = dve_ver_for("TRN2")
        s = DveOpSpec(name=name, opcode=row, uops=lower(spec, ver=ver),
                      rd1_en=_has_src1(spec))
        op = dve_ops.DveOp(name, spec, subdim=False,
                           uops_sha={ver: s.sha(ver)})
        dve_ops.OPS.append(op)
        dve_ops.CUSTOM_DVE_SPECS[name] = spec
        return op

    # C0=255, C1=256. C2/imm2 is unavailable with a 2-free-dim src1
    # (STT struct), so C1 doubles as the byte threshold and the *256
    # interleave multiplier.
    d = Bin(AluOp.ABSOLUTE_DIFF, Src0, Src1)
    e0 = Bin(AluOp.IS_NE, d, Zero) * Bin(AluOp.IS_NE, d, C1)
    c1 = Src1 >= C1
    even_body = e0 + Bin(AluOp.IS_NE, e0, c1) * C1

    e1 = d >= C0
    a0 = Bin(AluOp.IS_NE, Src0, Zero) * Bin(AluOp.IS_NE, Src0, C1)
    odd_body = e1 + Bin(AluOp.IS_NE, e1, a0) * C1

    def ref_even(in0, in1, s0, s1, imm2):
        v, w = in0.astype(np.int64), in1.astype(np.int64)
        dd = np.abs(v - w)
        e = ((dd != 0) & (dd != 256)).astype(np.int64)
        o = (e != (w >= 256)).astype(np.int64)
        return (e + 256 * o).astype(np.float32)

    def ref_odd(in0, in1, s0, s1, imm2):
        v, w = in0.astype(np.int64), in1.astype(np.int64)
        dd = np.abs(v - w)
        e = (dd >= 255).astype(np.int64)
        o = (e != ((v != 0) & (v != 256))).astype(np.int64)
        return (e + 256 * o).astype(np.float32)

    ev = register("CONV_ENC_EVEN", Spec(body=even_body, reference=ref_even))
    od = register("CONV_ENC_ODD", Spec(body=odd_body, reference=ref_odd))
    return ev, od


def _build_nc():
    import concourse.bass as bass  # noqa: F401
    import concourse.tile as tile
    from concourse import bacc, mybir

    ev, od = _register_ops()

    nc = bacc.Bacc(
        "TRN2",
        target_bir_lowering=False,
        debug=False,
        enable_asserts=False,
    )
    x = nc.dram_tensor("x", [SHARD_B, K], mybir.dt.uint8, kind="ExternalInput").ap()
    y = nc.dram_tensor(
        "y", [SHARD_B, N_OUT * K], mybir.dt.uint8, kind="ExternalOutput"
    ).ap()

    ZW = RPP * CHUNK + 4  # 4104 B/partition (u16/u32-aligned pad)
    H = K // 2            # 1024 u16 pairs per row

    with tile.TileContext(nc) as tc:
        with (
            tc.tile_pool(name="xin", bufs=1) as in_pool,
            tc.tile_pool(name="out", bufs=N_GROUPS) as out_pool,
        ):
            # Persistent input slots: [0,0,row_a | 0,0,row_b] per partition.
            # The 2-byte zero prefixes (encoder initial state) are written
            # once; per-iteration DMAs only fill the row payload bytes.
            slots = [
                in_pool.tile([P, ZW], mybir.dt.uint8, tag=f"xin{j}", name=f"xin{j}")
                for j in range(N_GROUPS)
            ]
            for j in range(N_GROUPS):
                nc.vector.memset(slots[j][:, 0:2], 0)
                nc.vector.memset(slots[j][:, CHUNK : CHUNK + 2], 0)

            for g in range(N_GROUPS):
                z = slots[g]
                rows = slice(g * ROWS_G, (g + 1) * ROWS_G)
                # src rows pair-packed: partition p <- rows (2p, 2p+1);
                # 4 KiB contiguous HBM read per partition.
                src = x[rows, :].rearrange("(p j) k -> p j k", j=RPP)
                dst = (
                    z[:, 2 : 2 + RPP * CHUNK]
                    .rearrange("p (j n) -> p j n", j=RPP)[:, :, 0:K]
                )
                nc.sync.dma_start(dst, src)

                z16 = z[:].bitcast(mybir.dt.uint16)  # [P, ZW/2]
                a16 = (
                    z16[:, 1 : 1 + RPP * (CHUNK // 2)]
                    .rearrange("p (j n) -> p j n", j=RPP)[:, :, 0:H]
                )
                c16 = (
                    z16[:, 0 : RPP * (CHUNK // 2)]
                    .rearrange("p (j n) -> p j n", j=RPP)[:, :, 0:H]
                )

                w = out_pool.tile(
                    [P, RPP * N_OUT * K], mybir.dt.uint8, tag="w", name="w"
                )
                w16 = (
                    w[:].bitcast(mybir.dt.uint16)
                    .rearrange("p (j n) -> p j n", j=RPP)  # [P, 2, 2048]
                )
                nc.vector._custom_dve(
                    ev, out=w16[:, :, 0 : N_OUT * H : 2], in0=a16, in1=c16,
                    s0=255.0, s1=256.0,
                )
                nc.vector._custom_dve(
                    od, out=w16[:, :, 1 : N_OUT * H : 2], in0=a16, in1=c16,
                    s0=255.0, s1=256.0,
                )

                dsty = y[rows, :].rearrange("(p j) k -> p j k", j=RPP)
                srcw = w[:].rearrange("p (j n) -> p j n", j=RPP)
                nc.gpsimd.dma_start(dsty, srcw)

    nc.compile()
    return nc


def _get_nc():
    if "nc" not in _compiled:
        _compiled["nc"] = _build_nc()
    return _compiled["nc"]


def kernel(**inputs) -> np.ndarray:
    from concourse.bass_utils import run_bass_kernel_spmd

    x_full = np.ascontiguousarray(
        np.asarray(inputs["inputs"]).astype(np.uint8)
    )
    assert x_full.shape == (B, K), x_full.shape

    nc = _get_nc()
    in_maps = [
        {"x": x_full[i * SHARD_B : (i + 1) * SHARD_B]} for i in range(N_CORES)
    ]
    res = run_bass_kernel_spmd(nc, in_maps, core_ids=list(range(N_CORES)))
    out = np.concatenate([r["y"] for r in res.results], axis=0)
    return out.astype(np.float32)
